# revision 10
# baseline (speedup 1.0000x reference)
"""Trainium2 Bass kernel for nn_Interpolator (quadratic-form kernel interpolation).

Math (T=8192 targets, C=8192 contexts, D=64, DY=32):
    S = W + W^T
    scores[t,c] = (z_t - z_c)^T W (z_t - z_c)
                = q_tt[t] + q_cc[c] - z_t^T S z_c
    theta = exp(-scores);  out = (theta @ y_context) / theta.sum(-1, keepdim)

q_tt[t] is a per-target factor on the whole theta row -> cancels in the
normalization -> dropped. q_cc[c] is a per-context factor folded into the
y-reduce weights on device: y'[c,:] = y_aug[c,:] * exp(-q_cc[c]), so
    theta' = exp(z_t^T S z_c)       (plain exp of the cross matmul)
    out2   = y'^T @ theta'          (identical product, fp-rounding aside)
This makes every matmul K=64 and frees the ACTIVATE of any per-partition bias.

Sharding: data-parallel over targets; each of 8 cores takes T/8 = 1024 targets
(2 passes x 512) and the full context set (64 chunks of 128).

Per-core engine plan (ACT is the roofline: 8.4M exps @ 1 lane-elem/cycle
@1.2GHz = 54.6us + ~260ns/instruction overhead):
  - every matmul is K=64 -> 64x128 PE row-tiles. Even chunks live on SBUF
    partitions 0-63 (tile T0), odd on 64-127 (T8); y-reduce splits each chunk
    into context halves lo->T0->o2a / hi->T8->o2b. Instructions on opposite
    tiles stream CONCURRENTLY and never share a PSUM bank, so cross, y, and
    q_cc matmuls all overlap and LDWEIGHTS loads on the idle tile.
  - pass 0: ACTIVATE N=1024 (2-chunk groups; sc 2x2 banks + zsn 2 + o2 2 = 8).
  - pass 1: zsn banks are free -> ACTIVATE N=1536 (3-slot groups, 2x3+2 = 8).
  - q_cc: zsn = -(1/2) zc S per chunk (PE, batched 8 same-parity chunks per
    PSUM bank), DVE mul against a natural-layout z slab + axis-X reduce
    -> Q[:,j] = -q_cc. One [128,64] exp -> EQ, one broadcast mul scales YA.
    y emission is delayed until EQ lands, then catches up 4 chunks/group --
    nearly free, because y half-matmuls ride the idle PE tile.
  - head: DMAs are priority-ordered across the 3 DMA queues (W/WT/ZT/LCO on
    sync, LCE on scalar, LCN/YA on gpsimd); N=128 filler matmuls bridge HAM
    warmup only until the RT chain is ready; RT is cast for pass-0 targets
    first so cross(0) starts the moment LCE lands.
Host: shard/transpose/cast/duplicate inputs (layout only), concat per-core
[33,1024] outputs, divide numerator rows by the denominator row.
"""

import ml_dtypes
import numpy as np

import concourse.bacc as bacc
import concourse.bass as bass
import concourse.mybir as mybir
import concourse.tile as tile
from concourse.bass_utils import run_bass_kernel_spmd

F32 = mybir.dt.float32
F16 = mybir.dt.float16
BF16 = mybir.dt.bfloat16

T, C, D, DY = 8192, 8192, 64, 32
NCORES = 8
TL = T // NCORES          # 1024 targets per core
TH = TL // 2              # 512 targets per pass
NCHUNK = C // 128         # 64 context chunks of 128
NG0 = 32                  # pass-0 groups (2 chunks each)
P1_GROUPS = [3] * 21 + [1]  # pass-1 slot grouping (sum 64)
EQ_GROUP = 12             # group after which EQ exp + YA scale emit
YCAP = 4                  # y catch-up chunks per emission step
NTH0 = 16                 # pass-0 theta ring
NTH1 = 8                  # pass-1 theta ring


def _build_kernel_body(tc: tile.TileContext):
    nc = tc.nc
    Exp = mybir.ActivationFunctionType.Exp
    Add = mybir.AluOpType.add
    X = mybir.AxisListType.X

    lce_d = nc.dram_tensor("lce", [D, NCHUNK // 2, 128], F16, kind="ExternalInput")
    lco_d = nc.dram_tensor("lco", [D, NCHUNK // 2, 128], F16, kind="ExternalInput")
    zcn_d = nc.dram_tensor("zcn", [128, NCHUNK, D], F16, kind="ExternalInput")
    ztd_d = nc.dram_tensor("ztd", [128, TL], F16, kind="ExternalInput")
    wd_d = nc.dram_tensor("wd", [128, D], F32, kind="ExternalInput")
    wtd_d = nc.dram_tensor("wtd", [128, D], F32, kind="ExternalInput")
    yad_d = nc.dram_tensor("yad", [128, DY, NCHUNK], BF16, kind="ExternalInput")
    out_d = nc.dram_tensor("out", [DY + 1, TL], F32, kind="ExternalOutput")

    with (
        tc.tile_pool(name="singles", bufs=1) as singles,
        tc.tile_pool(name="th0", bufs=NTH0) as thp0,
        tc.tile_pool(name="th1", bufs=NTH1) as thp1,
        tc.tile_pool(name="o2", bufs=1, space="PSUM") as o2p,
    ):
        # ---- resident SBUF slabs ----
        LCF = singles.tile([128, NCHUNK // 2, 128], F16, name="lcf")
        LCN = singles.tile([128, NCHUNK, D], F16, name="lcn")
        ZT = singles.tile([128, TL], F16, name="zt")
        RT = singles.tile([128, TL], F16, name="rt")
        WD = singles.tile([128, D], F32, name="wd")
        WTD = singles.tile([128, D], F32, name="wtd")
        SSF = singles.tile([128, D], F32, name="ssf")
        SS = singles.tile([128, D], F16, name="ss")
        SSQ = singles.tile([128, D], F16, name="ssq")
        P2 = singles.tile([128, 8, D], F32, name="p2")
        YA = singles.tile([128, DY + 1, NCHUNK], BF16, name="ya")
        Q = singles.tile([128, NCHUNK], F32, name="q")
        EQ = singles.tile([128, NCHUNK], F32, name="eq")
        OSB = singles.tile([DY + 1, TL], F32, name="osb")
        WRM = singles.tile([128, 128], BF16, name="wrm")
        EXD = singles.tile([128, 1], F32, name="exd")

        nc.vector.memset(WRM, 0.5)
        nc.vector.memset(EXD, 0.0)
        nc.scalar.activation(EXD, EXD, Exp)   # exp-table preload

        # o2a/o2b: per-pass numerator+denominator accumulators (1 bank each)
        o2a = [None, None]
        o2b = [None, None]

        with tc.tile_pool(name="warm", bufs=1, space="PSUM") as warmp:
            wps = warmp.tile([128, 512], F32, tag="warm")

            def fill(n):
                for _ in range(n):
                    nc.tensor.matmul(
                        wps[:, 0:128], WRM, WRM, start=True, stop=True
                    )

            fill(8)

            # priority-ordered loads on the 3 DMA queues
            nc.sync.dma_start(out=WD, in_=wd_d.ap())
            nc.sync.dma_start(out=WTD, in_=wtd_d.ap())
            nc.sync.dma_start(out=ZT, in_=ztd_d.ap())
            nc.scalar.dma_start(out=LCF[0:D, :, :], in_=lce_d.ap())
            nc.sync.dma_start(out=LCF[D:128, :, :], in_=lco_d.ap())
            nc.gpsimd.dma_start(out=LCN, in_=zcn_d.ap())
            nc.gpsimd.dma_start(out=YA[:, 0:DY, :], in_=yad_d.ap())

            nc.vector.tensor_add(SSF, WD, WTD)   # S = W + W^T (both halves)
            nc.vector.tensor_copy(SS, SSF)       # -> fp16
            nc.vector.tensor_scalar_mul(SSQ, SSF, -0.5)
            nc.vector.memset(YA[:, DY : DY + 1, :], 1.0)

            fill(6)

            # ---- prelude: RT = zsT = S ztT on both halves; pass-0 cols first
            with tc.tile_pool(name="prel", bufs=2, space="PSUM") as prelp:
                zpA = prelp.tile([128, TL], F32, tag="zp")
                zpB = prelp.tile([128, TL], F32, tag="zp")
                nc.tensor.matmul(zpA[0:D, 0:TH], SS[0:D, :], ZT[0:D, 0:TH],
                                 start=True, stop=True)
                nc.tensor.matmul(zpB[D:128, 0:TH], SS[D:128, :], ZT[D:128, 0:TH],
                                 start=True, stop=True)
                nc.tensor.matmul(zpA[0:D, TH:TL], SS[0:D, :], ZT[0:D, TH:TL],
                                 start=True, stop=True)
                nc.tensor.matmul(zpB[D:128, TH:TL], SS[D:128, :], ZT[D:128, TH:TL],
                                 start=True, stop=True)
                fill(8)
                nc.vector.tensor_copy(RT[0:D, 0:TH], zpA[0:D, 0:TH])
                nc.vector.tensor_copy(RT[D:128, 0:TH], zpB[D:128, 0:TH])
                nc.vector.tensor_copy(RT[0:D, TH:TL], zpA[0:D, TH:TL])
                nc.vector.tensor_copy(RT[D:128, TH:TL], zpB[D:128, TH:TL])
                fill(4)

        # ---- y-reduce emission machinery ----
        thref = [None] * NCHUNK  # chunk -> (th tile, col offset), current pass
        ynext = [0]              # next y step in 0..2*NCHUNK (pass*64 + chunk)

        def emit_y_chunk(s, p):
            if o2a[p] is None:
                o2a[p] = o2p.tile([DY + 1, TH], F32, tag="o2a", name=f"o2a{p}")
                o2b[p] = o2p.tile([DY + 1, TH], F32, tag="o2b", name=f"o2b{p}")
            th, col = thref[s]
            nc.tensor.matmul(
                o2a[p], YA[0:D, :, s], th[0:D, col : col + TH],
                start=(s == 0), stop=(s == NCHUNK - 1),
            )
            nc.tensor.matmul(
                o2b[p], YA[D:128, :, s], th[D:128, col : col + TH],
                start=(s == 0), stop=(s == NCHUNK - 1),
            )

        def flush(p):
            sl = slice(p * TH, (p + 1) * TH)
            nc.vector.tensor_copy(OSB[:, sl], o2a[p])
            nc.vector.tensor_add(OSB[:, sl], OSB[:, sl], o2b[p])

        def emit_y_steps(limit, budget):
            while budget and ynext[0] < limit:
                s = ynext[0]
                p, sch = divmod(s, NCHUNK)
                emit_y_chunk(sch, p)
                if sch == NCHUNK - 1:
                    flush(p)
                ynext[0] += 1
                budget -= 1

        def cross(s, p, dst):
            h = s & 1
            hp = slice(h * D, h * D + D)
            nc.tensor.matmul(
                dst, LCF[hp, s >> 1, :], RT[hp, p * TH : (p + 1) * TH],
                start=True, stop=True,
            )

        # ---- pass 0: 32 groups of 2 chunks; q_cc + EQ ride inside ----
        with (
            tc.tile_pool(name="sc0", bufs=2, space="PSUM") as scp0,
            tc.tile_pool(name="zsn", bufs=2, space="PSUM") as znp,
        ):
            zbatch = [None, None]  # parity -> current batch tile
            jq = [0]               # next q_cc chunk (E/O interleaved)

            def qcc_step():
                j = jq[0]
                if j >= NCHUNK:
                    return
                par = j & 1
                bi = (j >> 1) & 7
                if bi == 0:
                    zbatch[par] = znp.tile(
                        [128, 8, D], F32, tag="zsn", name=f"zb{j}"
                    )
                hp = slice(par * D, par * D + D)
                nc.tensor.matmul(
                    zbatch[par][:, bi, :], LCF[hp, j >> 1, :], SSQ[hp, :],
                    start=True, stop=True,
                )
                if bi == 7:
                    j0 = j - 14
                    nc.vector.tensor_mul(P2, zbatch[par], LCN[:, j0 : j0 + 15 : 2, :])
                    nc.vector.tensor_reduce(
                        Q[:, j0 : j0 + 15 : 2], P2, axis=X, op=Add
                    )
                jq[0] = j + 1

            for g in range(NG0):
                sc = scp0.tile([128, 2 * TH], F32, tag="sc")
                for k in range(2):
                    cross(2 * g + k, 0, sc[:, k * TH : (k + 1) * TH])
                th = thp0.tile([128, 2 * TH], BF16)
                nc.scalar.activation(th, sc, Exp)
                thref[2 * g] = (th, 0)
                thref[2 * g + 1] = (th, TH)
                for _ in range(8):
                    qcc_step()
                if g == EQ_GROUP:
                    nc.scalar.activation(EQ, Q, Exp)   # e^{-q_cc}
                    nc.vector.tensor_mul(
                        YA, YA, EQ[:, None, :].broadcast_to([128, DY + 1, NCHUNK])
                    )
                if g > EQ_GROUP:
                    emit_y_steps(2 * (g - 1), YCAP)

        # ---- pass 1: groups of 3 slots, ACT N=1536 ----
        with tc.tile_pool(name="sc1", bufs=2, space="PSUM") as scp1:
            s0 = 0
            for n in P1_GROUPS:
                sc = scp1.tile([128, 3 * TH], F32, tag="sc")
                for k in range(n):
                    cross(s0 + k, 1, sc[:, k * TH : (k + 1) * TH])
                th = thp1.tile([128, 3 * TH], BF16)
                nc.scalar.activation(th[:, 0 : n * TH], sc[:, 0 : n * TH], Exp)
                for k in range(n):
                    thref[s0 + k] = (th, k * TH)
                s0 += n
                emit_y_steps(NCHUNK + s0 - 2, YCAP)

            emit_y_steps(2 * NCHUNK, 2 * NCHUNK)
            nc.sync.dma_start(out=out_d.ap(), in_=OSB)


_CACHED = None


def _get_nc():
    global _CACHED
    if _CACHED is None:
        nc = bacc.Bacc(
            "TRN2",
            target_bir_lowering=False,
            debug=False,
            enable_asserts=False,
        )
        with tile.TileContext(nc) as tc:
            _build_kernel_body(tc)
        nc.compile()
        _CACHED = nc
    return _CACHED


def make_in_maps(z_context, y_context, z_target, W):
    """Host-side layout prep (transpose/reshape/cast/duplicate only) + shard."""
    z_context = np.asarray(z_context, dtype=np.float32)
    y_context = np.asarray(y_context, dtype=np.float32)
    z_target = np.asarray(z_target, dtype=np.float32)
    W = np.asarray(W, dtype=np.float32)

    zcT = z_context.T.astype(np.float16)               # [64, 8192]
    zc3 = zcT.reshape(D, NCHUNK, 128)
    lce = np.ascontiguousarray(zc3[:, 0::2, :])        # [64, 32, 128]
    lco = np.ascontiguousarray(zc3[:, 1::2, :])
    zcn = np.ascontiguousarray(
        z_context.reshape(NCHUNK, 128, D).transpose(1, 0, 2)
    ).astype(np.float16)                               # [128, 64, 64]
    yad = np.ascontiguousarray(
        y_context.reshape(NCHUNK, 128, DY).transpose(1, 2, 0)
    ).astype(ml_dtypes.bfloat16)                       # [128, 32, 64]
    wd = np.ascontiguousarray(np.concatenate([W, W], axis=0))       # [128, 64]
    wtd = np.ascontiguousarray(np.concatenate([W.T, W.T], axis=0))  # [128, 64]

    in_maps = []
    for i in range(NCORES):
        ztT = z_target[i * TL : (i + 1) * TL].T.astype(np.float16)  # [64, 1024]
        ztd = np.ascontiguousarray(np.concatenate([ztT, ztT], axis=0))
        in_maps.append(
            {"lce": lce, "lco": lco, "zcn": zcn, "ztd": ztd,
             "wd": wd, "wtd": wtd, "yad": yad}
        )
    return in_maps


def postprocess(results):
    """Gather per-core [33, TL] outputs -> full (T, DY) normalized output."""
    allT = np.concatenate([r["out"].T for r in results], axis=0)  # [T, 33]
    return (allT[:, :DY] / allT[:, DY : DY + 1]).astype(np.float32)


def run(in_maps, **kwargs):
    nc = _get_nc()
    return run_bass_kernel_spmd(nc, in_maps, core_ids=list(range(NCORES)), **kwargs)


def kernel(z_context, y_context, z_target, W):
    in_maps = make_in_maps(z_context, y_context, z_target, W)
    res = run(in_maps)
    return postprocess(res.results)


# revision 11
# speedup vs baseline: 1.1309x; 1.1309x over previous
"""Trainium2 Bass kernel for nn_Interpolator (quadratic-form kernel interpolation).

Math (T=8192 targets, C=8192 contexts, D=64, DY=32):
    S = W + W^T
    scores[t,c] = (z_t - z_c)^T W (z_t - z_c)
                = q_tt[t] + q_cc[c] - z_t^T S z_c
    theta = exp(-scores);  out = (theta @ y_context) / theta.sum(-1, keepdim)

q_tt[t] is a per-target factor on the whole theta row -> cancels in the
normalization -> dropped. q_cc[c] is a per-context factor folded into the
y-reduce weights on device: y'[c,:] = y_aug[c,:] * exp(-q_cc[c]), so
    theta' = exp(z_t^T S z_c)       (plain exp of the cross matmul)
    out2   = y'^T @ theta'          (identical product, fp-rounding aside)
This makes every matmul K=64 and frees the ACTIVATE of any per-partition bias.

Sharding: data-parallel over targets; each of 8 cores takes T/8 = 1024 targets
(2 passes x 512) and the full context set (64 chunks of 128).

Per-core engine plan (ACT is the roofline: 8.4M exps @ 1 lane-elem/cycle
@1.2GHz = 54.6us + ~260ns/instruction overhead):
  - every matmul is K=64 -> 64x128 PE row-tiles. Even chunks live on SBUF
    partitions 0-63 (tile T0), odd on 64-127 (T8); y-reduce splits each chunk
    into context halves lo->T0->o2a / hi->T8->o2b. Instructions on opposite
    tiles stream CONCURRENTLY and never share a PSUM bank, so cross, y, and
    q_cc matmuls all overlap and LDWEIGHTS loads on the idle tile.
  - pass 0: ACTIVATE N=1024 (2-chunk groups; sc 2x2 banks + zsn 2 + o2 2 = 8).
  - pass 1: zsn banks are free -> ACTIVATE N=1536 (3-slot groups, 2x3+2 = 8).
  - q_cc: zsn = -(1/2) zc S per chunk (PE, batched 8 same-parity chunks per
    PSUM bank), DVE mul against a natural-layout z slab + axis-X reduce
    -> Q[:,j] = -q_cc. One [128,64] exp -> EQ, one broadcast mul scales YA.
    y emission is delayed until EQ lands, then catches up 4 chunks/group --
    nearly free, because y half-matmuls ride the idle PE tile.
  - head: DMAs are priority-ordered across the 3 DMA queues (W/WT/ZT/LCO on
    sync, LCE on scalar, LCN/YA on gpsimd); N=128 filler matmuls bridge HAM
    warmup only until the RT chain is ready; RT is cast for pass-0 targets
    first so cross(0) starts the moment LCE lands.
Host: shard/transpose/cast/duplicate inputs (layout only), concat per-core
[33,1024] outputs, divide numerator rows by the denominator row.
"""

import ml_dtypes
import numpy as np

import concourse.bacc as bacc
import concourse.bass as bass
import concourse.mybir as mybir
import concourse.tile as tile
from concourse.bass_utils import run_bass_kernel_spmd

F32 = mybir.dt.float32
F16 = mybir.dt.float16
BF16 = mybir.dt.bfloat16

T, C, D, DY = 8192, 8192, 64, 32
NCORES = 8
TL = T // NCORES          # 1024 targets per core
TH = TL // 2              # 512 targets per pass
NCHUNK = C // 128         # 64 context chunks of 128
NG0 = 32                  # pass-0 groups (2 chunks each)
P1_GROUPS = [3] * 21 + [1]  # pass-1 slot grouping (sum 64)
EQ_GROUP = 11             # group after which EQ exp + YA scale emit
YCAP = 3                  # y catch-up chunks per emission step
NTH0 = 16                 # pass-0 theta ring
NTH1 = 8                  # pass-1 theta ring


def _build_kernel_body(tc: tile.TileContext):
    nc = tc.nc
    Exp = mybir.ActivationFunctionType.Exp
    Add = mybir.AluOpType.add
    X = mybir.AxisListType.X

    lce_d = nc.dram_tensor("lce", [D, NCHUNK // 2, 128], F16, kind="ExternalInput")
    lco_d = nc.dram_tensor("lco", [D, NCHUNK // 2, 128], F16, kind="ExternalInput")
    zcn_d = nc.dram_tensor("zcn", [128, NCHUNK, D], F16, kind="ExternalInput")
    ztd_d = nc.dram_tensor("ztd", [D, TL], F16, kind="ExternalInput")
    wpk_d = nc.dram_tensor("wpk", [D, 128], F16, kind="ExternalInput")
    yad_d = nc.dram_tensor("yad", [128, DY, NCHUNK], BF16, kind="ExternalInput")
    out_d = nc.dram_tensor("out", [DY + 1, TL], F32, kind="ExternalOutput")

    with (
        tc.tile_pool(name="singles", bufs=1) as singles,
        tc.tile_pool(name="th0", bufs=NTH0) as thp0,
        tc.tile_pool(name="th1", bufs=NTH1) as thp1,
        tc.tile_pool(name="o2", bufs=1, space="PSUM") as o2p,
    ):
        # ---- resident SBUF slabs ----
        LCF = singles.tile([128, NCHUNK // 2, 128], F16, name="lcf")
        LCN = singles.tile([128, NCHUNK, D], F16, name="lcn")
        ZT = singles.tile([D, TL], F16, name="zt")
        RT = singles.tile([128, TL], F16, name="rt")
        WP = singles.tile([D, 128], F16, name="wp")
        SQT = singles.tile([D, D], F16, name="sqt")
        SSQ = singles.tile([128, D], F16, name="ssq")
        P2 = singles.tile([128, 8, D], F32, name="p2")
        YA = singles.tile([128, DY + 1, NCHUNK], BF16, name="ya")
        Q = singles.tile([128, NCHUNK], F32, name="q")
        EQ = singles.tile([128, NCHUNK], F32, name="eq")
        OSB = singles.tile([DY + 1, TL], F32, name="osb")
        WRM = singles.tile([128, 128], BF16, name="wrm")
        EXD = singles.tile([128, 1], F32, name="exd")

        nc.vector.memset(WRM, 0.5)
        nc.vector.memset(EXD, 0.0)
        nc.scalar.activation(EXD, EXD, Exp)   # exp-table preload

        # o2a/o2b: per-pass numerator+denominator accumulators (1 bank each)
        o2a = [None, None]
        o2b = [None, None]

        with tc.tile_pool(name="warm", bufs=1, space="PSUM") as warmp:
            wps = warmp.tile([128, 512], F32, tag="warm")

            def fill(n):
                for _ in range(n):
                    nc.tensor.matmul(
                        wps[:, 0:128], WRM, WRM, start=True, stop=True
                    )

            fill(6)

            # priority-ordered loads; dup-copies ride the scalar queue so a
            # dependent SBUF->SBUF DMA never blocks an HBM transfer behind it
            nc.sync.dma_start(out=WP, in_=wpk_d.ap())
            nc.sync.dma_start(out=ZT, in_=ztd_d.ap())
            nc.scalar.dma_start(out=LCF[0:D, :, :], in_=lce_d.ap())
            nc.sync.dma_start(out=LCF[D:128, :, :], in_=lco_d.ap())
            nc.gpsimd.dma_start(out=LCN, in_=zcn_d.ap())
            nc.gpsimd.dma_start(out=YA[:, 0:DY, :], in_=yad_d.ap())

            # SSQ = -(W + W^T)/2 in fp16; hi half duplicated on-chip
            nc.vector.tensor_add(SQT, WP[:, 0:D], WP[:, D:128])
            nc.vector.tensor_scalar_mul(SSQ[0:D, :], SQT, -0.5)
            nc.scalar.dma_start(out=SSQ[D:128, :], in_=SSQ[0:D, :])
            nc.vector.memset(YA[:, DY : DY + 1, :], 1.0)

            # ---- prelude: RT = S ztT via two accumulating matmuls (lhsT=W
            # gives W^T zt, lhsT=W^T gives W zt); pass-0 target half first,
            # T8 rows via on-chip dup DMA off the critical path.
            with tc.tile_pool(name="prel", bufs=2, space="PSUM") as prelp:
                zpA = prelp.tile([128, TL], F32, tag="zp")
                for ph in range(2):
                    sl = slice(ph * TH, (ph + 1) * TH)
                    nc.tensor.matmul(zpA[0:D, sl], WP[:, 0:D], ZT[:, sl],
                                     start=True, stop=False)
                    nc.tensor.matmul(zpA[0:D, sl], WP[:, D:128], ZT[:, sl],
                                     start=False, stop=True)
                    nc.vector.tensor_copy(RT[0:D, sl], zpA[0:D, sl])
                    nc.scalar.dma_start(out=RT[D:128, sl], in_=RT[0:D, sl])

        # ---- y-reduce emission machinery ----
        thref = [None] * NCHUNK  # chunk -> (th tile, col offset), current pass
        ynext = [0]              # next y step in 0..2*NCHUNK (pass*64 + chunk)

        def emit_y_chunk(s, p):
            if o2a[p] is None:
                o2a[p] = o2p.tile([DY + 1, TH], F32, tag="o2a", name=f"o2a{p}")
                o2b[p] = o2p.tile([DY + 1, TH], F32, tag="o2b", name=f"o2b{p}")
            th, col = thref[s]
            nc.tensor.matmul(
                o2a[p], YA[0:D, :, s], th[0:D, col : col + TH],
                start=(s == 0), stop=(s == NCHUNK - 1),
            )
            nc.tensor.matmul(
                o2b[p], YA[D:128, :, s], th[D:128, col : col + TH],
                start=(s == 0), stop=(s == NCHUNK - 1),
            )

        def flush(p):
            sl = slice(p * TH, (p + 1) * TH)
            nc.vector.tensor_copy(OSB[:, sl], o2a[p])
            nc.vector.tensor_add(OSB[:, sl], OSB[:, sl], o2b[p])

        def emit_y_steps(limit, budget):
            while budget and ynext[0] < limit:
                s = ynext[0]
                p, sch = divmod(s, NCHUNK)
                emit_y_chunk(sch, p)
                if sch == NCHUNK - 1:
                    flush(p)
                ynext[0] += 1
                budget -= 1

        def cross(s, p, dst):
            h = s & 1
            hp = slice(h * D, h * D + D)
            nc.tensor.matmul(
                dst, LCF[hp, s >> 1, :], RT[hp, p * TH : (p + 1) * TH],
                start=True, stop=True,
            )

        # ---- pass 0: 32 groups of 2 chunks; q_cc + EQ ride inside ----
        with (
            tc.tile_pool(name="sc0", bufs=2, space="PSUM") as scp0,
            tc.tile_pool(name="zsn", bufs=2, space="PSUM") as znp,
        ):
            zbatch = [None, None]  # parity -> current batch tile
            jq = [0]               # next q_cc chunk (E/O interleaved)

            def qcc_step():
                j = jq[0]
                if j >= NCHUNK:
                    return
                par = j & 1
                bi = (j >> 1) & 7
                if bi == 0:
                    zbatch[par] = znp.tile(
                        [128, 8, D], F32, tag="zsn", name=f"zb{j}"
                    )
                hp = slice(par * D, par * D + D)
                nc.tensor.matmul(
                    zbatch[par][:, bi, :], LCF[hp, j >> 1, :], SSQ[hp, :],
                    start=True, stop=True,
                )
                if bi == 7:
                    j0 = j - 14
                    nc.vector.tensor_mul(P2, zbatch[par], LCN[:, j0 : j0 + 15 : 2, :])
                    nc.vector.tensor_reduce(
                        Q[:, j0 : j0 + 15 : 2], P2, axis=X, op=Add
                    )
                jq[0] = j + 1

            for g in range(NG0):
                sc = scp0.tile([128, 2 * TH], F32, tag="sc")
                for k in range(2):
                    cross(2 * g + k, 0, sc[:, k * TH : (k + 1) * TH])
                th = thp0.tile([128, 2 * TH], BF16)
                nc.scalar.activation(th, sc, Exp)
                thref[2 * g] = (th, 0)
                thref[2 * g + 1] = (th, TH)
                if g >= 2:
                    for _ in range(8):
                        qcc_step()
                if g == EQ_GROUP:
                    nc.scalar.activation(EQ, Q, Exp)   # e^{-q_cc}
                    nc.vector.tensor_mul(
                        YA, YA, EQ[:, None, :].broadcast_to([128, DY + 1, NCHUNK])
                    )
                if g > EQ_GROUP:
                    emit_y_steps(2 * g, YCAP)

        # ---- pass 1: groups of 3 slots, ACT N=1536 ----
        with tc.tile_pool(name="sc1", bufs=2, space="PSUM") as scp1:
            s0 = 0
            for n in P1_GROUPS:
                sc = scp1.tile([128, 3 * TH], F32, tag="sc")
                for k in range(n):
                    cross(s0 + k, 1, sc[:, k * TH : (k + 1) * TH])
                th = thp1.tile([128, 3 * TH], BF16)
                nc.scalar.activation(th[:, 0 : n * TH], sc[:, 0 : n * TH], Exp)
                for k in range(n):
                    thref[s0 + k] = (th, k * TH)
                s0 += n
                emit_y_steps(NCHUNK + s0 - n, YCAP)

            emit_y_steps(2 * NCHUNK, 2 * NCHUNK)
            nc.sync.dma_start(out=out_d.ap(), in_=OSB)


_CACHED = None


def _get_nc():
    global _CACHED
    if _CACHED is None:
        nc = bacc.Bacc(
            "TRN2",
            target_bir_lowering=False,
            debug=False,
            enable_asserts=False,
        )
        with tile.TileContext(nc) as tc:
            _build_kernel_body(tc)
        nc.compile()
        _CACHED = nc
    return _CACHED


def make_in_maps(z_context, y_context, z_target, W):
    """Host-side layout prep (transpose/reshape/cast/duplicate only) + shard."""
    z_context = np.asarray(z_context, dtype=np.float32)
    y_context = np.asarray(y_context, dtype=np.float32)
    z_target = np.asarray(z_target, dtype=np.float32)
    W = np.asarray(W, dtype=np.float32)

    zcT = z_context.T.astype(np.float16)               # [64, 8192]
    zc3 = zcT.reshape(D, NCHUNK, 128)
    lce = np.ascontiguousarray(zc3[:, 0::2, :])        # [64, 32, 128]
    lco = np.ascontiguousarray(zc3[:, 1::2, :])
    zcn = np.ascontiguousarray(
        z_context.reshape(NCHUNK, 128, D).transpose(1, 0, 2)
    ).astype(np.float16)                               # [128, 64, 64]
    yad = np.ascontiguousarray(
        y_context.reshape(NCHUNK, 128, DY).transpose(1, 2, 0)
    ).astype(ml_dtypes.bfloat16)                       # [128, 32, 64]
    wpk = np.ascontiguousarray(
        np.concatenate([W, W.T], axis=1)
    ).astype(np.float16)                               # [64, 128] = [W | W^T]

    in_maps = []
    for i in range(NCORES):
        ztd = np.ascontiguousarray(
            z_target[i * TL : (i + 1) * TL].T.astype(np.float16)
        )                                              # [64, 1024]
        in_maps.append(
            {"lce": lce, "lco": lco, "zcn": zcn, "ztd": ztd,
             "wpk": wpk, "yad": yad}
        )
    return in_maps


def postprocess(results):
    """Gather per-core [33, TL] outputs -> full (T, DY) normalized output."""
    allT = np.concatenate([r["out"].T for r in results], axis=0)  # [T, 33]
    return (allT[:, :DY] / allT[:, DY : DY + 1]).astype(np.float32)


def run(in_maps, **kwargs):
    nc = _get_nc()
    return run_bass_kernel_spmd(nc, in_maps, core_ids=list(range(NCORES)), **kwargs)


def kernel(z_context, y_context, z_target, W):
    in_maps = make_in_maps(z_context, y_context, z_target, W)
    res = run(in_maps)
    return postprocess(res.results)


# revision 12
# speedup vs baseline: 1.1459x; 1.0132x over previous
"""Trainium2 Bass kernel for nn_Interpolator (quadratic-form kernel interpolation).

Math (T=8192 targets, C=8192 contexts, D=64, DY=32):
    S = W + W^T
    scores[t,c] = (z_t - z_c)^T W (z_t - z_c)
                = q_tt[t] + q_cc[c] - z_t^T S z_c
    theta = exp(-scores);  out = (theta @ y_context) / theta.sum(-1, keepdim)

q_tt[t] is a per-target factor on the whole theta row -> cancels in the
normalization -> dropped. q_cc[c] is a per-context factor folded into the
y-reduce weights on device: y'[c,:] = y_aug[c,:] * exp(-q_cc[c]), so
    theta' = exp(z_t^T S z_c)       (plain exp of the cross matmul)
    out2   = y'^T @ theta'          (identical product, fp-rounding aside)
This makes every matmul K=64 and frees the ACTIVATE of any per-partition bias.

Sharding: data-parallel over targets; each of 8 cores takes T/8 = 1024 targets
(2 passes x 512) and the full context set (64 chunks of 128).

Per-core engine plan (ACT is the roofline: 8.4M exps @ 1 lane-elem/cycle
@1.2GHz = 54.6us + ~260ns/instruction overhead):
  - every matmul is K=64 -> 64x128 PE row-tiles. Even chunks live on SBUF
    partitions 0-63 (tile T0), odd on 64-127 (T8); y-reduce splits each chunk
    into context halves lo->T0->o2a / hi->T8->o2b. Instructions on opposite
    tiles stream CONCURRENTLY and never share a PSUM bank, so cross, y, and
    q_cc matmuls all overlap and LDWEIGHTS loads on the idle tile.
  - pass 0: ACTIVATE N=1024 (2-chunk groups; sc 2x2 banks + zsn 2 + o2 2 = 8).
  - pass 1: zsn banks are free -> ACTIVATE N=1536 (3-slot groups, 2x3+2 = 8).
  - q_cc: zsn = -(1/2) zc S per chunk (PE, batched 8 same-parity chunks per
    PSUM bank), DVE mul against a natural-layout z slab + axis-X reduce
    -> Q[:,j] = -q_cc. One [128,64] exp -> EQ, one broadcast mul scales YA.
    y emission is delayed until EQ lands, then catches up 4 chunks/group --
    nearly free, because y half-matmuls ride the idle PE tile.
  - head: DMAs are priority-ordered across the 3 DMA queues (W/WT/ZT/LCO on
    sync, LCE on scalar, LCN/YA on gpsimd); N=128 filler matmuls bridge HAM
    warmup only until the RT chain is ready; RT is cast for pass-0 targets
    first so cross(0) starts the moment LCE lands.
Host: shard/transpose/cast/duplicate inputs (layout only), concat per-core
[33,1024] outputs, divide numerator rows by the denominator row.
"""

import ml_dtypes
import numpy as np

import concourse.bacc as bacc
import concourse.bass as bass
import concourse.mybir as mybir
import concourse.tile as tile
from concourse.bass_utils import run_bass_kernel_spmd

F32 = mybir.dt.float32
F16 = mybir.dt.float16
BF16 = mybir.dt.bfloat16

T, C, D, DY = 8192, 8192, 64, 32
NCORES = 8
TL = T // NCORES          # 1024 targets per core
TH = TL // 2              # 512 targets per pass
NCHUNK = C // 128         # 64 context chunks of 128
NG0 = 32                  # pass-0 groups (2 chunks each)
P1_GROUPS = [3] * 21 + [1]  # pass-1 slot grouping (sum 64)
EQ_GROUP = 11             # group after which EQ exp + YA scale emit
YCAP0 = 2                 # pass-0 y catch-up chunks per step
YCAP1 = 4                 # pass-1 y catch-up chunks per step
NTH0 = 16                 # pass-0 theta ring
NTH1 = 8                  # pass-1 theta ring


def _build_kernel_body(tc: tile.TileContext):
    nc = tc.nc
    Exp = mybir.ActivationFunctionType.Exp
    Add = mybir.AluOpType.add
    X = mybir.AxisListType.X

    lce_d = nc.dram_tensor("lce", [D, NCHUNK // 2, 128], F16, kind="ExternalInput")
    lco_d = nc.dram_tensor("lco", [D, NCHUNK // 2, 128], F16, kind="ExternalInput")
    zcn_d = nc.dram_tensor("zcn", [128, NCHUNK, D], F16, kind="ExternalInput")
    ztd_d = nc.dram_tensor("ztd", [128, TL], F16, kind="ExternalInput")
    wpk_d = nc.dram_tensor("wpk", [128, 128], F16, kind="ExternalInput")
    yad_d = nc.dram_tensor("yad", [128, DY, NCHUNK], BF16, kind="ExternalInput")
    out_d = nc.dram_tensor("out", [DY + 1, TL], F32, kind="ExternalOutput")

    with (
        tc.tile_pool(name="singles", bufs=1) as singles,
        tc.tile_pool(name="th0", bufs=NTH0) as thp0,
        tc.tile_pool(name="th1", bufs=NTH1) as thp1,
        tc.tile_pool(name="o2", bufs=1, space="PSUM") as o2p,
    ):
        # ---- resident SBUF slabs ----
        LCF = singles.tile([128, NCHUNK // 2, 128], F16, name="lcf")
        LCN = singles.tile([128, NCHUNK, D], F16, name="lcn")
        ZT = singles.tile([128, TL], F16, name="zt")
        RT = singles.tile([128, TL], F16, name="rt")
        WP = singles.tile([128, 128], F16, name="wp")
        SQT = singles.tile([128, D], F16, name="sqt")
        SSQ = singles.tile([128, D], F16, name="ssq")
        P2 = singles.tile([128, 8, D], F32, name="p2")
        YA = singles.tile([128, DY + 1, NCHUNK], BF16, name="ya")
        Q = singles.tile([128, NCHUNK], F32, name="q")
        EQ = singles.tile([128, NCHUNK], F32, name="eq")
        OSB = singles.tile([DY + 1, TL], F32, name="osb")
        WRM = singles.tile([128, 128], BF16, name="wrm")
        EXD = singles.tile([128, 1], F32, name="exd")

        nc.vector.memset(WRM, 0.5)
        nc.vector.memset(EXD, 0.0)
        nc.scalar.activation(EXD, EXD, Exp)   # exp-table preload

        # o2a/o2b: per-pass numerator+denominator accumulators (1 bank each)
        o2a = [None, None]
        o2b = [None, None]

        with tc.tile_pool(name="warm", bufs=1, space="PSUM") as warmp:
            wps = warmp.tile([128, 512], F32, tag="warm")

            def fill(n):
                for _ in range(n):
                    nc.tensor.matmul(
                        wps[:, 0:128], WRM, WRM, start=True, stop=True
                    )

            fill(6)

            # priority-ordered loads: critical small tensors lead each queue
            nc.scalar.dma_start(out=WP, in_=wpk_d.ap())
            nc.sync.dma_start(out=ZT, in_=ztd_d.ap())
            nc.scalar.dma_start(out=LCF[0:D, :, :], in_=lce_d.ap())
            nc.sync.dma_start(out=LCF[D:128, :, :], in_=lco_d.ap())
            nc.gpsimd.dma_start(out=LCN, in_=zcn_d.ap())
            nc.gpsimd.dma_start(out=YA[:, 0:DY, :], in_=yad_d.ap())

            # SSQ = -(W + W^T)/2 in fp16 (both halves, inputs pre-duplicated)
            nc.vector.tensor_add(SQT, WP[:, 0:D], WP[:, D:128])
            nc.vector.tensor_scalar_mul(SSQ, SQT, -0.5)
            nc.vector.memset(YA[:, DY : DY + 1, :], 1.0)

            # ---- prelude: RT = S ztT via two accumulating matmuls per tile
            # (lhsT=W gives W^T zt, lhsT=W^T gives W zt); pass-0 half first.
            with tc.tile_pool(name="prel", bufs=2, space="PSUM") as prelp:
                zpA = prelp.tile([128, TL], F32, tag="zp")
                zpB = prelp.tile([128, TL], F32, tag="zp")
                for ph in range(2):
                    sl = slice(ph * TH, (ph + 1) * TH)
                    nc.tensor.matmul(zpA[0:D, sl], WP[0:D, 0:D], ZT[0:D, sl],
                                     start=True, stop=False)
                    nc.tensor.matmul(zpB[D:128, sl], WP[D:128, 0:D], ZT[D:128, sl],
                                     start=True, stop=False)
                    nc.tensor.matmul(zpA[0:D, sl], WP[0:D, D:128], ZT[0:D, sl],
                                     start=False, stop=True)
                    nc.tensor.matmul(zpB[D:128, sl], WP[D:128, D:128], ZT[D:128, sl],
                                     start=False, stop=True)
                    nc.vector.tensor_copy(RT[0:D, sl], zpA[0:D, sl])
                    nc.vector.tensor_copy(RT[D:128, sl], zpB[D:128, sl])

        # ---- y-reduce emission machinery ----
        thref = [None] * NCHUNK  # chunk -> (th tile, col offset), current pass
        ynext = [0]              # next y step in 0..2*NCHUNK (pass*64 + chunk)

        def emit_y_chunk(s, p):
            if o2a[p] is None:
                o2a[p] = o2p.tile([DY + 1, TH], F32, tag="o2a", name=f"o2a{p}")
                o2b[p] = o2p.tile([DY + 1, TH], F32, tag="o2b", name=f"o2b{p}")
            th, col = thref[s]
            nc.tensor.matmul(
                o2a[p], YA[0:D, :, s], th[0:D, col : col + TH],
                start=(s == 0), stop=(s == NCHUNK - 1),
            )
            nc.tensor.matmul(
                o2b[p], YA[D:128, :, s], th[D:128, col : col + TH],
                start=(s == 0), stop=(s == NCHUNK - 1),
            )

        def flush(p):
            sl = slice(p * TH, (p + 1) * TH)
            nc.vector.tensor_copy(OSB[:, sl], o2a[p])
            nc.vector.tensor_add(OSB[:, sl], OSB[:, sl], o2b[p])
            eng = nc.sync if p == 0 else nc.scalar
            eng.dma_start(out=out_d.ap()[:, sl], in_=OSB[:, sl])

        def emit_y_steps(limit, budget):
            while budget and ynext[0] < limit:
                s = ynext[0]
                p, sch = divmod(s, NCHUNK)
                emit_y_chunk(sch, p)
                if sch == NCHUNK - 1:
                    flush(p)
                ynext[0] += 1
                budget -= 1

        def cross(s, p, dst):
            h = s & 1
            hp = slice(h * D, h * D + D)
            nc.tensor.matmul(
                dst, LCF[hp, s >> 1, :], RT[hp, p * TH : (p + 1) * TH],
                start=True, stop=True,
            )

        # ---- pass 0: 32 groups of 2 chunks; q_cc + EQ ride inside ----
        with (
            tc.tile_pool(name="sc0", bufs=2, space="PSUM") as scp0,
            tc.tile_pool(name="zsn", bufs=2, space="PSUM") as znp,
        ):
            zbatch = [None, None]  # parity -> current batch tile
            jq = [0]               # next q_cc chunk (E/O interleaved)

            def qcc_step():
                j = jq[0]
                if j >= NCHUNK:
                    return
                par = j & 1
                bi = (j >> 1) & 7
                if bi == 0:
                    zbatch[par] = znp.tile(
                        [128, 8, D], F32, tag="zsn", name=f"zb{j}"
                    )
                hp = slice(par * D, par * D + D)
                nc.tensor.matmul(
                    zbatch[par][:, bi, :], LCF[hp, j >> 1, :], SSQ[hp, :],
                    start=True, stop=True,
                )
                if bi == 7:
                    j0 = j - 14
                    nc.vector.tensor_mul(P2, zbatch[par], LCN[:, j0 : j0 + 15 : 2, :])
                    nc.vector.tensor_reduce(
                        Q[:, j0 : j0 + 15 : 2], P2, axis=X, op=Add
                    )
                jq[0] = j + 1

            for g in range(NG0):
                sc = scp0.tile([128, 2 * TH], F32, tag="sc")
                for k in range(2):
                    cross(2 * g + k, 0, sc[:, k * TH : (k + 1) * TH])
                th = thp0.tile([128, 2 * TH], BF16)
                nc.scalar.activation(th, sc, Exp)
                thref[2 * g] = (th, 0)
                thref[2 * g + 1] = (th, TH)
                if g >= 2:
                    for _ in range(8):
                        qcc_step()
                if g == EQ_GROUP:
                    nc.scalar.activation(EQ, Q, Exp)   # e^{-q_cc}
                    nc.vector.tensor_mul(
                        YA, YA, EQ[:, None, :].broadcast_to([128, DY + 1, NCHUNK])
                    )
                if g > EQ_GROUP:
                    emit_y_steps(2 * g, YCAP0)

        # ---- pass 1: groups of 3 slots, ACT N=1536 ----
        with tc.tile_pool(name="sc1", bufs=2, space="PSUM") as scp1:
            s0 = 0
            for n in P1_GROUPS:
                sc = scp1.tile([128, 3 * TH], F32, tag="sc")
                for k in range(n):
                    cross(s0 + k, 1, sc[:, k * TH : (k + 1) * TH])
                th = thp1.tile([128, 3 * TH], BF16)
                nc.scalar.activation(th[:, 0 : n * TH], sc[:, 0 : n * TH], Exp)
                for k in range(n):
                    thref[s0 + k] = (th, k * TH)
                s0 += n
                emit_y_steps(NCHUNK + s0 - n, YCAP1)

            emit_y_steps(2 * NCHUNK, 2 * NCHUNK)


_CACHED = None


def _get_nc():
    global _CACHED
    if _CACHED is None:
        nc = bacc.Bacc(
            "TRN2",
            target_bir_lowering=False,
            debug=False,
            enable_asserts=False,
        )
        with tile.TileContext(nc) as tc:
            _build_kernel_body(tc)
        nc.compile()
        _CACHED = nc
    return _CACHED


def make_in_maps(z_context, y_context, z_target, W):
    """Host-side layout prep (transpose/reshape/cast/duplicate only) + shard."""
    z_context = np.asarray(z_context, dtype=np.float32)
    y_context = np.asarray(y_context, dtype=np.float32)
    z_target = np.asarray(z_target, dtype=np.float32)
    W = np.asarray(W, dtype=np.float32)

    zcT = z_context.T.astype(np.float16)               # [64, 8192]
    zc3 = zcT.reshape(D, NCHUNK, 128)
    lce = np.ascontiguousarray(zc3[:, 0::2, :])        # [64, 32, 128]
    lco = np.ascontiguousarray(zc3[:, 1::2, :])
    zcn = np.ascontiguousarray(
        z_context.reshape(NCHUNK, 128, D).transpose(1, 0, 2)
    ).astype(np.float16)                               # [128, 64, 64]
    yad = np.ascontiguousarray(
        y_context.reshape(NCHUNK, 128, DY).transpose(1, 2, 0)
    ).astype(ml_dtypes.bfloat16)                       # [128, 32, 64]
    wp1 = np.concatenate([W, W.T], axis=1).astype(np.float16)  # [64,128]=[W|W^T]
    wpk = np.ascontiguousarray(np.concatenate([wp1, wp1], axis=0))  # dup rows

    in_maps = []
    for i in range(NCORES):
        zt1 = z_target[i * TL : (i + 1) * TL].T.astype(np.float16)  # [64, 1024]
        ztd = np.ascontiguousarray(np.concatenate([zt1, zt1], axis=0))
        in_maps.append(
            {"lce": lce, "lco": lco, "zcn": zcn, "ztd": ztd,
             "wpk": wpk, "yad": yad}
        )
    return in_maps


def postprocess(results):
    """Gather per-core [33, TL] outputs -> full (T, DY) normalized output."""
    allT = np.concatenate([r["out"].T for r in results], axis=0)  # [T, 33]
    return (allT[:, :DY] / allT[:, DY : DY + 1]).astype(np.float32)


def run(in_maps, **kwargs):
    nc = _get_nc()
    return run_bass_kernel_spmd(nc, in_maps, core_ids=list(range(NCORES)), **kwargs)


def kernel(z_context, y_context, z_target, W):
    in_maps = make_in_maps(z_context, y_context, z_target, W)
    res = run(in_maps)
    return postprocess(res.results)


# revision 13
# speedup vs baseline: 1.1590x; 1.0115x over previous
"""Trainium2 Bass kernel for nn_Interpolator (quadratic-form kernel interpolation).

Math (T=8192 targets, C=8192 contexts, D=64, DY=32):
    S = W + W^T
    scores[t,c] = (z_t - z_c)^T W (z_t - z_c)
                = q_tt[t] + q_cc[c] - z_t^T S z_c
    theta = exp(-scores);  out = (theta @ y_context) / theta.sum(-1, keepdim)

q_tt[t] is a per-target factor on the whole theta row -> cancels in the
normalization -> dropped. q_cc[c] is a per-context factor folded into the
y-reduce weights on device: y'[c,:] = y_aug[c,:] * exp(-q_cc[c]), so
    theta' = exp(z_t^T S z_c)       (plain exp of the cross matmul)
    out2   = y'^T @ theta'          (identical product, fp-rounding aside)
This makes every matmul K=64 and frees the ACTIVATE of any per-partition bias.

Sharding: data-parallel over targets; each of 8 cores takes T/8 = 1024 targets
(2 passes x 512) and the full context set (64 chunks of 128).

Per-core engine plan (ACT is the roofline: 8.4M exps @ 1 lane-elem/cycle
@1.2GHz = 54.6us + ~260ns/instruction overhead):
  - every matmul is K=64 -> 64x128 PE row-tiles. Even chunks live on SBUF
    partitions 0-63 (tile T0), odd on 64-127 (T8); y-reduce splits each chunk
    into context halves lo->T0->o2a / hi->T8->o2b. Instructions on opposite
    tiles stream CONCURRENTLY and never share a PSUM bank, so cross, y, and
    q_cc matmuls all overlap and LDWEIGHTS loads on the idle tile.
  - pass 0: ACTIVATE N=1024 (2-chunk groups; sc 2x2 banks + zsn 2 + o2 2 = 8).
  - pass 1: zsn banks are free -> ACTIVATE N=1536 (3-slot groups, 2x3+2 = 8).
  - q_cc: zsn = -(1/2) zc S per chunk (PE, batched 8 same-parity chunks per
    PSUM bank), DVE mul against a natural-layout z slab + axis-X reduce
    -> Q[:,j] = -q_cc. One [128,64] exp -> EQ, one broadcast mul scales YA.
    y emission is delayed until EQ lands, then catches up 4 chunks/group --
    nearly free, because y half-matmuls ride the idle PE tile.
  - head: DMAs are priority-ordered across the 3 DMA queues (W/WT/ZT/LCO on
    sync, LCE on scalar, LCN/YA on gpsimd); N=128 filler matmuls bridge HAM
    warmup only until the RT chain is ready; RT is cast for pass-0 targets
    first so cross(0) starts the moment LCE lands.
Host: shard/transpose/cast/duplicate inputs (layout only), concat per-core
[33,1024] outputs, divide numerator rows by the denominator row.
"""

import ml_dtypes
import numpy as np

import concourse.bacc as bacc
import concourse.bass as bass
import concourse.mybir as mybir
import concourse.tile as tile
from concourse.bass_utils import run_bass_kernel_spmd

F32 = mybir.dt.float32
F16 = mybir.dt.float16
BF16 = mybir.dt.bfloat16

T, C, D, DY = 8192, 8192, 64, 32
NCORES = 8
TL = T // NCORES          # 1024 targets per core
TH = TL // 2              # 512 targets per pass
NCHUNK = C // 128         # 64 context chunks of 128
NG0 = 32                  # pass-0 groups (2 chunks each)
P1_GROUPS = [3] * 21 + [1]  # pass-1 slot grouping (sum 64)
EQ_GROUP = 13             # group after which EQ exp + YA scale emit
YCAP0 = 2                 # pass-0 y catch-up chunks per step
YCAP1 = 4                 # pass-1 y catch-up chunks per step
NTH0 = 16                 # pass-0 theta ring
NTH1 = 8                  # pass-1 theta ring


def _build_kernel_body(tc: tile.TileContext):
    nc = tc.nc
    Exp = mybir.ActivationFunctionType.Exp
    Add = mybir.AluOpType.add
    X = mybir.AxisListType.X

    lce_d = nc.dram_tensor("lce", [D, NCHUNK // 2, 128], F16, kind="ExternalInput")
    lco_d = nc.dram_tensor("lco", [D, NCHUNK // 2, 128], F16, kind="ExternalInput")
    zcn_d = nc.dram_tensor("zcn", [128, NCHUNK, D], F16, kind="ExternalInput")
    ztd_d = nc.dram_tensor("ztd", [128, TL], F16, kind="ExternalInput")
    wpk_d = nc.dram_tensor("wpk", [128, 128], F16, kind="ExternalInput")
    yad_d = nc.dram_tensor("yad", [128, DY, NCHUNK], BF16, kind="ExternalInput")
    out_d = nc.dram_tensor("out", [DY + 1, TL], F32, kind="ExternalOutput")

    with (
        tc.tile_pool(name="singles", bufs=1) as singles,
        tc.tile_pool(name="th0", bufs=NTH0) as thp0,
        tc.tile_pool(name="th1", bufs=NTH1) as thp1,
        tc.tile_pool(name="o2", bufs=1, space="PSUM") as o2p,
    ):
        # ---- resident SBUF slabs ----
        LCF = singles.tile([128, NCHUNK // 2, 128], F16, name="lcf")
        LCN = singles.tile([128, NCHUNK, D], F16, name="lcn")
        ZT = singles.tile([128, TL], F16, name="zt")
        RT = singles.tile([128, TL], F16, name="rt")
        WP = singles.tile([128, 128], F16, name="wp")
        SQT = singles.tile([128, D], F16, name="sqt")
        SSQ = singles.tile([128, D], F16, name="ssq")
        P2 = singles.tile([128, 8, D], F32, name="p2")
        YA = singles.tile([128, DY + 1, NCHUNK], BF16, name="ya")
        Q = singles.tile([128, NCHUNK], F32, name="q")
        EQ = singles.tile([128, NCHUNK], F32, name="eq")
        OSB = singles.tile([DY + 1, TL], F32, name="osb")
        WRM = singles.tile([128, 128], BF16, name="wrm")
        EXD = singles.tile([128, 1], F32, name="exd")

        nc.vector.memset(WRM, 0.5)
        nc.vector.memset(EXD, 0.0)
        nc.scalar.activation(EXD, EXD, Exp)   # exp-table preload

        # o2a/o2b: per-pass numerator+denominator accumulators (1 bank each)
        o2a = [None, None]
        o2b = [None, None]

        with tc.tile_pool(name="warm", bufs=1, space="PSUM") as warmp:
            wps = warmp.tile([128, 512], F32, tag="warm")

            def fill(n):
                for _ in range(n):
                    nc.tensor.matmul(
                        wps[:, 0:128], WRM, WRM, start=True, stop=True
                    )

            fill(6)

            # priority-ordered loads; big slabs split into need-ordered
            # sub-DMAs so the stream starts after the first slice lands
            nc.scalar.dma_start(out=WP, in_=wpk_d.ap())
            nc.sync.dma_start(out=ZT[:, 0:TH], in_=ztd_d.ap()[:, 0:TH])
            Q8 = NCHUNK // 8  # 8 chunk-pairs per slab slice
            for q in range(4):
                qs_ = slice(q * Q8, (q + 1) * Q8)
                nc.scalar.dma_start(out=LCF[0:D, qs_, :], in_=lce_d.ap()[:, qs_, :])
            nc.sync.dma_start(out=LCF[D:128, 0:Q8, :], in_=lco_d.ap()[:, 0:Q8, :])
            nc.sync.dma_start(out=ZT[:, TH:TL], in_=ztd_d.ap()[:, TH:TL])
            for q in range(1, 4):
                qs_ = slice(q * Q8, (q + 1) * Q8)
                nc.sync.dma_start(out=LCF[D:128, qs_, :], in_=lco_d.ap()[:, qs_, :])
            for q in range(4):
                qs_ = slice(q * 16, (q + 1) * 16)
                nc.gpsimd.dma_start(out=LCN[:, qs_, :], in_=zcn_d.ap()[:, qs_, :])
            nc.gpsimd.dma_start(out=YA[:, 0:DY, :], in_=yad_d.ap())

            # SSQ = -(W + W^T)/2 in fp16 (both halves, inputs pre-duplicated)
            nc.vector.tensor_add(SQT, WP[:, 0:D], WP[:, D:128])
            nc.vector.tensor_scalar_mul(SSQ, SQT, -0.5)
            nc.vector.memset(YA[:, DY : DY + 1, :], 1.0)

            # ---- prelude: RT = S ztT via two accumulating matmuls per tile
            # (lhsT=W gives W^T zt, lhsT=W^T gives W zt); pass-0 half first.
            with tc.tile_pool(name="prel", bufs=2, space="PSUM") as prelp:
                zpA = prelp.tile([128, TL], F32, tag="zp")
                zpB = prelp.tile([128, TL], F32, tag="zp")
                for ph in range(2):
                    sl = slice(ph * TH, (ph + 1) * TH)
                    nc.tensor.matmul(zpA[0:D, sl], WP[0:D, 0:D], ZT[0:D, sl],
                                     start=True, stop=False)
                    nc.tensor.matmul(zpB[D:128, sl], WP[D:128, 0:D], ZT[D:128, sl],
                                     start=True, stop=False)
                    nc.tensor.matmul(zpA[0:D, sl], WP[0:D, D:128], ZT[0:D, sl],
                                     start=False, stop=True)
                    nc.tensor.matmul(zpB[D:128, sl], WP[D:128, D:128], ZT[D:128, sl],
                                     start=False, stop=True)
                    nc.vector.tensor_copy(RT[0:D, sl], zpA[0:D, sl])
                    nc.vector.tensor_copy(RT[D:128, sl], zpB[D:128, sl])

        # ---- y-reduce emission machinery ----
        thref = [None] * NCHUNK  # chunk -> (th tile, col offset), current pass
        ynext = [0]              # next y step in 0..2*NCHUNK (pass*64 + chunk)

        def emit_y_chunk(s, p):
            if o2a[p] is None:
                o2a[p] = o2p.tile([DY + 1, TH], F32, tag="o2a", name=f"o2a{p}")
                o2b[p] = o2p.tile([DY + 1, TH], F32, tag="o2b", name=f"o2b{p}")
            th, col = thref[s]
            nc.tensor.matmul(
                o2a[p], YA[0:D, :, s], th[0:D, col : col + TH],
                start=(s == 0), stop=(s == NCHUNK - 1),
            )
            nc.tensor.matmul(
                o2b[p], YA[D:128, :, s], th[D:128, col : col + TH],
                start=(s == 0), stop=(s == NCHUNK - 1),
            )

        def flush(p):
            sl = slice(p * TH, (p + 1) * TH)
            nc.vector.tensor_copy(OSB[:, sl], o2a[p])
            nc.vector.tensor_add(OSB[:, sl], OSB[:, sl], o2b[p])
            eng = nc.sync if p == 0 else nc.gpsimd
            eng.dma_start(out=out_d.ap()[:, sl], in_=OSB[:, sl])

        def emit_y_steps(limit, budget):
            while budget and ynext[0] < limit:
                s = ynext[0]
                p, sch = divmod(s, NCHUNK)
                emit_y_chunk(sch, p)
                if sch == NCHUNK - 1:
                    flush(p)
                ynext[0] += 1
                budget -= 1

        def cross(s, p, dst):
            h = s & 1
            hp = slice(h * D, h * D + D)
            nc.tensor.matmul(
                dst, LCF[hp, s >> 1, :], RT[hp, p * TH : (p + 1) * TH],
                start=True, stop=True,
            )

        # ---- pass 0: 32 groups of 2 chunks; q_cc + EQ ride inside ----
        with (
            tc.tile_pool(name="sc0", bufs=2, space="PSUM") as scp0,
            tc.tile_pool(name="zsn", bufs=2, space="PSUM") as znp,
        ):
            zbatch = [None, None]  # parity -> current batch tile
            jq = [0]               # next q_cc chunk (E/O interleaved)

            def qcc_step():
                j = jq[0]
                if j >= NCHUNK:
                    return
                par = j & 1
                bi = (j >> 1) & 7
                if bi == 0:
                    zbatch[par] = znp.tile(
                        [128, 8, D], F32, tag="zsn", name=f"zb{j}"
                    )
                hp = slice(par * D, par * D + D)
                nc.tensor.matmul(
                    zbatch[par][:, bi, :], LCF[hp, j >> 1, :], SSQ[hp, :],
                    start=True, stop=True,
                )
                if bi == 7:
                    j0 = j - 14
                    nc.vector.tensor_mul(P2, zbatch[par], LCN[:, j0 : j0 + 15 : 2, :])
                    nc.vector.tensor_reduce(
                        Q[:, j0 : j0 + 15 : 2], P2, axis=X, op=Add
                    )
                jq[0] = j + 1

            for g in range(NG0):
                sc = scp0.tile([128, 2 * TH], F32, tag="sc")
                for k in range(2):
                    cross(2 * g + k, 0, sc[:, k * TH : (k + 1) * TH])
                th = thp0.tile([128, 2 * TH], BF16)
                nc.scalar.activation(th, sc, Exp)
                thref[2 * g] = (th, 0)
                thref[2 * g + 1] = (th, TH)
                if g >= 2:
                    for _ in range(8):
                        qcc_step()
                if g == EQ_GROUP:
                    nc.scalar.activation(EQ, Q, Exp)   # e^{-q_cc}
                    nc.vector.tensor_mul(
                        YA, YA, EQ[:, None, :].broadcast_to([128, DY + 1, NCHUNK])
                    )
                if g > EQ_GROUP:
                    emit_y_steps(2 * g, YCAP0)

        # ---- pass 1: groups of 3 slots, ACT N=1536 ----
        with tc.tile_pool(name="sc1", bufs=2, space="PSUM") as scp1:
            s0 = 0
            for n in P1_GROUPS:
                sc = scp1.tile([128, 3 * TH], F32, tag="sc")
                for k in range(n):
                    cross(s0 + k, 1, sc[:, k * TH : (k + 1) * TH])
                th = thp1.tile([128, 3 * TH], BF16)
                nc.scalar.activation(th[:, 0 : n * TH], sc[:, 0 : n * TH], Exp)
                for k in range(n):
                    thref[s0 + k] = (th, k * TH)
                s0 += n
                emit_y_steps(NCHUNK + s0 - n, YCAP1)

            emit_y_steps(2 * NCHUNK, 2 * NCHUNK)


_CACHED = None


def _get_nc():
    global _CACHED
    if _CACHED is None:
        nc = bacc.Bacc(
            "TRN2",
            target_bir_lowering=False,
            debug=False,
            enable_asserts=False,
        )
        with tile.TileContext(nc) as tc:
            _build_kernel_body(tc)
        nc.compile()
        _CACHED = nc
    return _CACHED


def make_in_maps(z_context, y_context, z_target, W):
    """Host-side layout prep (transpose/reshape/cast/duplicate only) + shard."""
    z_context = np.asarray(z_context, dtype=np.float32)
    y_context = np.asarray(y_context, dtype=np.float32)
    z_target = np.asarray(z_target, dtype=np.float32)
    W = np.asarray(W, dtype=np.float32)

    zcT = z_context.T.astype(np.float16)               # [64, 8192]
    zc3 = zcT.reshape(D, NCHUNK, 128)
    lce = np.ascontiguousarray(zc3[:, 0::2, :])        # [64, 32, 128]
    lco = np.ascontiguousarray(zc3[:, 1::2, :])
    zcn = np.ascontiguousarray(
        z_context.reshape(NCHUNK, 128, D).transpose(1, 0, 2)
    ).astype(np.float16)                               # [128, 64, 64]
    yad = np.ascontiguousarray(
        y_context.reshape(NCHUNK, 128, DY).transpose(1, 2, 0)
    ).astype(ml_dtypes.bfloat16)                       # [128, 32, 64]
    wp1 = np.concatenate([W, W.T], axis=1).astype(np.float16)  # [64,128]=[W|W^T]
    wpk = np.ascontiguousarray(np.concatenate([wp1, wp1], axis=0))  # dup rows

    in_maps = []
    for i in range(NCORES):
        zt1 = z_target[i * TL : (i + 1) * TL].T.astype(np.float16)  # [64, 1024]
        ztd = np.ascontiguousarray(np.concatenate([zt1, zt1], axis=0))
        in_maps.append(
            {"lce": lce, "lco": lco, "zcn": zcn, "ztd": ztd,
             "wpk": wpk, "yad": yad}
        )
    return in_maps


def postprocess(results):
    """Gather per-core [33, TL] outputs -> full (T, DY) normalized output."""
    allT = np.concatenate([r["out"].T for r in results], axis=0)  # [T, 33]
    return (allT[:, :DY] / allT[:, DY : DY + 1]).astype(np.float32)


def run(in_maps, **kwargs):
    nc = _get_nc()
    return run_bass_kernel_spmd(nc, in_maps, core_ids=list(range(NCORES)), **kwargs)


def kernel(z_context, y_context, z_target, W):
    in_maps = make_in_maps(z_context, y_context, z_target, W)
    res = run(in_maps)
    return postprocess(res.results)
